# revision 15
# baseline (speedup 1.0000x reference)
"""CRF forward-algorithm loss + argmax decode on 8 TRN2 NeuronCores.

Math: the CRF forward scan over T=65536 steps is a chain of 2x2
log-semiring matrix products (associative). Each core handles 8 batch
elements, computes the first 5 levels of a parallel tree reduction over
the chain on-chip (emissions laid out [128 partitions x 4096]), and
exports 128 partial-product matrices per partition row. The host
finishes the reduction in float64 and assembles the loss. decoded is an
elementwise compare. alpha0 is absorbed into the t=1 leaf as a column
offset so the whole chain reduces to one logsumexp over the final 2x2.

Stored matrices carry compile-time scalar offsets (K) that fold the
transition-matrix constants into ACT softplus biases; offsets are
restored on the host.
"""

import os
import sys

import numpy as np

if "/opt/trn_rl_repo" not in sys.path:
    sys.path.insert(0, "/opt/trn_rl_repo")

import concourse.bacc as bacc
import concourse.bass as bass
import concourse.mybir as mybir
import concourse.tile as tile
from concourse.bass_utils import run_bass_kernel_spmd

F32 = mybir.dt.float32
I32 = mybir.dt.int32

B, C, H, W = 64, 2, 256, 256
T = H * W                  # 65536
NCORE = 8
BLOC = B // NCORE          # 8 batch elements per core
NCHUNK = 16                # chunks per batch element -> 8*16 = 128 partitions
CH = T // NCHUNK           # 4096 t-positions per partition row
NLVL = 5                   # tree levels computed on chip
FOUT = CH >> NLVL          # 128 partial products per partition row exported

_BUILD_CACHE = {}

# consts tensor layout: [0:4] level-1 softplus biases D1[j][i],
# [4:8] fixup consts, [8:8+4*(NLVL-1)] per-level bK biases.
NCONST = 8 + 4 * (NLVL - 1)


def _tr_consts(tr):
    """Per-level constants mirroring the on-chip offset tracking.

    Returns (consts[NCONST] float32, K_final[2,2] float64).
    """
    tr = np.asarray(tr, dtype=np.float64)
    cst = np.zeros(NCONST, np.float64)
    for j in range(2):
        for i in range(2):
            cst[2 * j + i] = (tr[j, 1] - tr[j, 0]) + (tr[1, i] - tr[0, i])
            cst[4 + 2 * j + i] = tr[j, i] - tr[j, 0] - tr[0, i]
    K = [[tr[j, 0] + tr[0, i] for i in range(2)] for j in range(2)]
    for lvl in range(2, NLVL + 1):
        off = 8 + 4 * (lvl - 2)
        for j in range(2):
            for i in range(2):
                cst[off + 2 * j + i] = (K[j][1] - K[j][0]) + (K[1][i] - K[0][i])
        K = [[K[j][0] + K[0][i] for i in range(2)] for j in range(2)]
    return cst.astype(np.float32), np.array(K)


def _build():
    """Build (and cache) the single-core Bass program. Transition-derived
    scalars arrive at runtime via the small "cst" input tensor."""
    if "nc" in _BUILD_CACHE:
        return _BUILD_CACHE["nc"]

    nc = bacc.Bacc("TRN2", target_bir_lowering=False, debug=False,
                   num_devices=NCORE)
    logits_d = nc.dram_tensor("logits", [BLOC, 2, T], F32, kind="ExternalInput")
    cst_d = nc.dram_tensor("cst", [128, NCONST], F32, kind="ExternalInput")
    dec_d = nc.dram_tensor("decoded", [BLOC, T], I32, kind="ExternalOutput")
    ptail_d = nc.dram_tensor("ptail", [128, 4 * FOUT], F32, kind="ExternalOutput")

    gt = mybir.AluOpType.is_gt
    add = mybir.AluOpType.add
    # softplus(z) = ln(1 + exp(z)); exp+ln share one ACT table set.
    EXP_FN = mybir.ActivationFunctionType.Exp
    LN_FN = mybir.ActivationFunctionType.Ln

    with tile.TileContext(nc) as tc:
        with tc.tile_pool(name="main", bufs=1) as pool:
            cst = pool.tile([128, NCONST], F32, tag="cst")
            nc.sync.dma_start(cst[:, :], cst_d.ap())

            def bias_ap(k, parts=128):
                return cst[0:parts, k:k + 1]

            E0 = pool.tile([128, CH], F32, tag="e0")
            E1 = pool.tile([128, CH], F32, tag="e1")
            src = logits_d.ap().rearrange("b c (k f) -> b c k f", f=CH)
            nc.sync.dma_start(E0[:, :], src[:, 0])
            nc.sync.dma_start(E1[:, :], src[:, 1])

            # decoded = argmax over classes = (e1 > e0)
            dec = pool.tile([128, CH], I32, tag="dec")
            nc.vector.tensor_tensor(dec[:, :], E1[:, :], E0[:, :], op=gt)
            ddst = dec_d.ap().rearrange("b (k f) -> b k f", f=CH)
            nc.sync.dma_start(ddst, dec[:, :])

            # ---- level 1: pair matrices straight from emissions ----
            Fh = CH // 2  # 2048
            de = pool.tile([128, Fh], F32, tag="de")
            nc.vector.tensor_sub(de[:, :], E1[:, 0:CH:2], E0[:, 0:CH:2])
            L = pool.tile([128, 4 * Fh], F32, tag="st")
            for j in range(2):
                for i in range(2):
                    blk = 2 * j + i
                    ls = slice(blk * Fh, (blk + 1) * Fh)
                    nc.scalar.activation(L[:, ls], de[:, :], EXP_FN,
                                         bias=bias_ap(blk))
                    nc.scalar.activation(L[:, ls], L[:, ls], LN_FN, bias=1.0)
            tt = pool.tile([128, 2 * Fh], F32, tag="tt")
            nc.gpsimd.tensor_add(tt[:, 0:Fh], E0[:, 1:CH:2], E0[:, 0:CH:2])
            nc.gpsimd.tensor_add(tt[:, Fh:2 * Fh], E1[:, 1:CH:2], E0[:, 0:CH:2])
            C1 = pool.tile([128, 4 * Fh], F32, tag="ca")
            for j in range(2):
                for i in range(2):
                    blk = 2 * j + i
                    nc.vector.tensor_add(C1[:, blk * Fh:(blk + 1) * Fh],
                                         L[:, blk * Fh:(blk + 1) * Fh],
                                         tt[:, j * Fh:(j + 1) * Fh])

            # ---- fixup: leaf 0 := identity, absorb alpha0 into t=1 leaf ----
            # C1[p=b*16, pos 0] = e1[j] + e0[i] + (tr[j,i]-tr[j,0]-tr[0,i])
            V = pool.tile([8, 4], F32, tag="vfix")
            nc.sync.dma_start(V[:, 0:2], E0[0:128:16, 0:2])
            nc.sync.dma_start(V[:, 2:4], E1[0:128:16, 0:2])
            Fx = pool.tile([8, 4], F32, tag="ffix")
            for j in range(2):
                for i in range(2):
                    blk = 2 * j + i
                    nc.vector.scalar_tensor_tensor(
                        Fx[:, blk:blk + 1], V[:, 2 * j + 1:2 * j + 2],
                        bias_ap(4 + blk, parts=8), V[:, 2 * i:2 * i + 1],
                        op0=add, op1=add)
            for j in range(2):
                for i in range(2):
                    blk = 2 * j + i
                    nc.sync.dma_start(C1[0:128:16, blk * Fh:blk * Fh + 1],
                                      Fx[:, blk:blk + 1])

            # ---- levels 2..NLVL: generic log-semiring combine ----
            Cin = C1
            N = Fh  # positions per partition at current level input
            ca_next = ["cb", "ca", "cb", "ca"]
            for lvl in range(2, NLVL + 1):
                F = N // 2
                coff = 8 + 4 * (lvl - 2)
                Xt = pool.tile([128, 4 * F], F32, tag="xt")
                Yt = pool.tile([128, 4 * F], F32, tag="yt")
                St = pool.tile([128, 4 * F], F32, tag="st")
                Ct = pool.tile([128, 4 * F], F32, tag=ca_next[lvl - 2])

                def blkap(t, blk, par, n=N):
                    off = blk * n
                    return t[:, off + par:off + n:2]

                for j in range(2):
                    for i in range(2):
                        blk = 2 * j + i
                        xs = slice(blk * F, (blk + 1) * F)
                        # X = A[j,0] + B[0,i]   (A odd, B even)
                        nc.vector.tensor_add(Xt[:, xs],
                                             blkap(Cin, 2 * j, 1),
                                             blkap(Cin, i, 0))
                        # Y = A[j,1] + B[1,i]
                        nc.gpsimd.tensor_add(Yt[:, xs],
                                             blkap(Cin, 2 * j + 1, 1),
                                             blkap(Cin, 2 + i, 0))
                        # D = Y - X (in place on Y)
                        nc.vector.tensor_sub(Yt[:, xs], Yt[:, xs], Xt[:, xs])
                        # SP = softplus(D + bK) = ln(1 + exp(D + bK))
                        nc.scalar.activation(St[:, xs], Yt[:, xs], EXP_FN,
                                             bias=bias_ap(coff + blk))
                        nc.scalar.activation(St[:, xs], St[:, xs], LN_FN,
                                             bias=1.0)
                        # C = X + SP
                        nc.vector.tensor_add(Ct[:, xs], Xt[:, xs], St[:, xs])
                Cin = Ct
                N = F

            nc.sync.dma_start(ptail_d.ap(), Cin[:, :])

    nc.compile()
    _BUILD_CACHE["nc"] = nc
    return nc


def _host_tail(ptails, K_final):
    """Finish the reduction in float64. ptails: [NCORE, 128, 4*FOUT]."""
    m = ptails.astype(np.float64).reshape(NCORE * BLOC, NCHUNK, 2, 2, FOUT)
    m = np.moveaxis(m, 4, 2).reshape(B, NCHUNK * FOUT, 2, 2)
    m = m + K_final[None, None]
    # fold the ordered chain with a balanced tree (vectorized over b)
    seq = m
    while seq.shape[1] > 1:
        n = seq.shape[1]
        if n % 2:
            carry = seq[:, -1:]
            seq = seq[:, :-1]
        else:
            carry = None
        Bm = seq[:, 0::2]   # earlier
        Am = seq[:, 1::2]   # later
        out = np.empty_like(Am)
        for j in range(2):
            for i in range(2):
                out[..., j, i] = np.logaddexp(
                    Am[..., j, 0] + Bm[..., 0, i],
                    Am[..., j, 1] + Bm[..., 1, i])
        if carry is not None:
            # fold the trailing (latest) element onto the last pair product
            last = out[:, -1:]
            merged = np.empty_like(last)
            for j in range(2):
                for i in range(2):
                    merged[..., j, i] = np.logaddexp(
                        carry[..., j, 0] + last[..., 0, i],
                        carry[..., j, 1] + last[..., 1, i])
            out[:, -1:] = merged
        seq = out
    P = seq[:, 0]  # [B, 2, 2]
    LL = np.logaddexp.reduce(P.reshape(B, 4), axis=1)
    return -(LL.sum() / B)


def run(logits, mask, transitions, trace=False):
    logits = np.ascontiguousarray(np.asarray(logits), dtype=np.float32)
    tr = np.ascontiguousarray(np.asarray(transitions), dtype=np.float32)
    cvals, K_final = _tr_consts(tr)
    cst = np.ascontiguousarray(np.broadcast_to(cvals, (128, NCONST)))
    nc = _build()
    shards = logits.reshape(NCORE, BLOC, 2, T)
    in_maps = [{"logits": np.ascontiguousarray(shards[c]), "cst": cst}
               for c in range(NCORE)]
    res = run_bass_kernel_spmd(nc, in_maps, list(range(NCORE)), trace=trace)
    ptails = np.stack([res.results[c]["ptail"] for c in range(NCORE)])
    loss = np.float32(_host_tail(ptails, K_final))
    dec = np.concatenate([res.results[c]["decoded"] for c in range(NCORE)])
    decoded = dec.reshape(-1, 1).astype(np.int32)
    return (loss, decoded), res


def kernel(logits, mask, transitions):
    out, _ = run(logits, mask, transitions, trace=False)
    return out


# revision 16
# speedup vs baseline: 1.3624x; 1.3624x over previous
"""CRF forward-algorithm loss + argmax decode on 8 TRN2 NeuronCores.

Math: the CRF forward scan over T=65536 steps is a chain of 2x2
log-semiring matrix products (associative). Each core handles 8 batch
elements, computes the first NLVL levels of a parallel tree reduction
over the chain on-chip (emissions laid out [128 partitions x 4096]),
and exports the partial-product matrices. The host finishes the
reduction in float64 and assembles the loss. decoded is an elementwise
compare. alpha0 is absorbed into the t=1 leaf as a column offset so the
whole chain reduces to one logsumexp over the final 2x2 per batch.

Implementation notes (measured on HW):
- GpSimd elementwise runs concurrently with VectorE at ~1/3 speed and
  slows VectorE ~3x (shared SBUF ports) -> all TT work stays on VectorE,
  transcendentals on ScalarE (they coexist at full speed).
- Levels >= 2 combine in exp-domain: C = ln(exp(X-d) + exp(Y+bK-d)),
  one DVE add for S instead of D-sub + C-add, with per-level centering
  constants d (estimated from a data sample) keeping exp in range.
- exp and ln share one ACT table set; the table-load pass is patched so
  only natural_log_exp_and_others supplies them (avoids 24 table loads).
- Transition-derived scalars arrive via the small "cst" input, so the
  compiled program is input-independent and cached across calls.
"""

import os
import sys
import types

import numpy as np

if "/opt/trn_rl_repo" not in sys.path:
    sys.path.insert(0, "/opt/trn_rl_repo")

import concourse.bacc as bacc
import concourse.bass as bass
import concourse.mybir as mybir
import concourse.tile as tile
from concourse.bass_utils import run_bass_kernel_spmd

F32 = mybir.dt.float32
I32 = mybir.dt.int32

B, C, H, W = 64, 2, 256, 256
T = H * W                  # 65536
NCORE = 8
BLOC = B // NCORE          # 8 batch elements per core
NCHUNK = 16                # chunks per batch element -> 8*16 = 128 partitions
CH = T // NCHUNK           # 4096 t-positions per partition row
NLVL = 5                   # tree levels computed on chip
FOUT = CH >> NLVL          # level-NLVL positions per partition row exported

_BUILD_CACHE = {}

# consts layout: [0:4] level-1 exp biases D1[j][i], [4:8] fixup consts,
# then per level l=2..NLVL 5 entries: [4 EY biases (bK-d), 1 EX bias (-d)].
NCONST = 8 + 5 * (NLVL - 1)


def _consts(tr, deltas):
    """Constants mirroring the on-chip offset tracking.

    tr: [2,2] transitions; deltas: per-level centering d_2..d_NLVL.
    Returns (consts[NCONST] float32, K_final[2,2] float64).
    """
    tr = np.asarray(tr, dtype=np.float64)
    cst = np.zeros(NCONST, np.float64)
    for j in range(2):
        for i in range(2):
            cst[2 * j + i] = (tr[j, 1] - tr[j, 0]) + (tr[1, i] - tr[0, i])
            cst[4 + 2 * j + i] = tr[j, i] - tr[j, 0] - tr[0, i]
    K = [[tr[j, 0] + tr[0, i] for i in range(2)] for j in range(2)]
    for lvl in range(2, NLVL + 1):
        off = 8 + 5 * (lvl - 2)
        d = float(deltas[lvl - 2])
        for j in range(2):
            for i in range(2):
                bK = (K[j][1] - K[j][0]) + (K[1][i] - K[0][i])
                cst[off + 2 * j + i] = bK - d
        cst[off + 4] = -d
        K = [[K[j][0] + K[0][i] + d for i in range(2)] for j in range(2)]
    return cst.astype(np.float32), np.array(K)


def _estimate_deltas(logits, tr):
    """Per-level centering constants from a small sample (float32 sim of
    the on-chip tree on the first 8 batch elements, 512 cols/partition)."""
    f32 = np.float32
    tr = np.asarray(tr, np.float64)
    lv = logits[0:BLOC].reshape(BLOC, 2, NCHUNK, CH)[:, :, :, 0:512]
    E0 = lv[:, 0].reshape(128, 512).astype(f32)
    E1 = lv[:, 1].reshape(128, 512).astype(f32)

    def sp(x):
        return np.logaddexp(0.0, x.astype(np.float64)).astype(f32)

    de = E1[:, 0::2] - E0[:, 0::2]
    Cb = {}
    K = [[tr[j, 0] + tr[0, i] for i in range(2)] for j in range(2)]
    for j in range(2):
        for i in range(2):
            D = (tr[j, 1] - tr[j, 0]) + (tr[1, i] - tr[0, i])
            t = (E0[:, 1::2] + E0[:, 0::2]) if j == 0 else (E1[:, 1::2] + E0[:, 0::2])
            Cb[(j, i)] = (t + sp(de + f32(D))).astype(f32)
    deltas = []
    for lvl in range(2, NLVL + 1):
        X00 = Cb[(0, 0)][:, 1::2] + Cb[(0, 0)][:, 0::2]
        d = float(np.round(X00.mean()))
        deltas.append(d)
        Xn = {}
        for j in range(2):
            for i in range(2):
                Xv = (Cb[(2 * j // 2, 0)][:, 1::2] + Cb[(0, i)][:, 0::2]).astype(f32)
                # careful: A-block (j,0) odd, B-block (0,i) even
                Xv = (Cb[(j, 0)][:, 1::2] + Cb[(0, i)][:, 0::2]).astype(f32)
                Yv = (Cb[(j, 1)][:, 1::2] + Cb[(1, i)][:, 0::2]).astype(f32)
                bK = (K[j][1] - K[j][0]) + (K[1][i] - K[0][i])
                S = np.exp(Xv - d) + np.exp(Yv + f32(bK - d))
                Xn[(j, i)] = np.log(S).astype(f32)
        K = [[K[j][0] + K[0][i] + d for i in range(2)] for j in range(2)]
        Cb = Xn
    return deltas


def _patched_act_tables(self):
    """insert_act_table_loads with Exp/Ln pinned to the one set that has
    both, so exp->ln sequences never thrash table loads."""
    import bass_rust
    from concourse.hw_specs import get_activation_tables
    EXP = mybir.ActivationFunctionType.Exp
    LN = mybir.ActivationFunctionType.Ln
    keep = "natural_log_exp_and_others"
    tables = [
        (name, funcs if name == keep
         else {f for f in funcs if f not in (EXP, LN)})
        for name, funcs in get_activation_tables(self.m.arch).items()
    ]
    bass_rust.insert_act_table_loads(self, tables)


def _build():
    if "nc" in _BUILD_CACHE:
        return _BUILD_CACHE["nc"]

    nc = bacc.Bacc("TRN2", target_bir_lowering=False, debug=False,
                   num_devices=NCORE)
    nc.insert_act_table_loads = types.MethodType(_patched_act_tables, nc)
    logits_d = nc.dram_tensor("logits", [BLOC, 2, T], F32, kind="ExternalInput")
    cst_d = nc.dram_tensor("cst", [128, NCONST], F32, kind="ExternalInput")
    dec_d = nc.dram_tensor("decoded", [BLOC, T], I32, kind="ExternalOutput")
    ptail_d = nc.dram_tensor("ptail", [128, 4 * FOUT], F32, kind="ExternalOutput")

    gt = mybir.AluOpType.is_gt
    add = mybir.AluOpType.add
    EXP_FN = mybir.ActivationFunctionType.Exp
    LN_FN = mybir.ActivationFunctionType.Ln

    with tile.TileContext(nc) as tc:
        with tc.tile_pool(name="main", bufs=1) as pool:
            cst = pool.tile([128, NCONST], F32, tag="cst")
            nc.sync.dma_start(cst[:, :], cst_d.ap())

            def bias_ap(k, parts=128):
                return cst[0:parts, k:k + 1]

            E0 = pool.tile([128, CH], F32, tag="e0")
            E1 = pool.tile([128, CH], F32, tag="e1")
            src = logits_d.ap().rearrange("b c (k f) -> b c k f", f=CH)
            # two HWDGE rings in parallel: sync + scalar issuers
            nc.sync.dma_start(E0[:, :], src[:, 0])
            nc.scalar.dma_start(E1[:, :], src[:, 1])

            # decoded = argmax over classes = (e1 > e0)
            dec = pool.tile([128, CH], I32, tag="dec")
            nc.vector.tensor_tensor(dec[:, :], E1[:, :], E0[:, :], op=gt)
            ddst = dec_d.ap().rearrange("b (k f) -> b k f", f=CH)
            nc.sync.dma_start(ddst, dec[:, :])

            # ---- level 1: pair matrices straight from emissions ----
            Fh = CH // 2  # 2048
            de = pool.tile([128, Fh], F32, tag="de")
            nc.vector.tensor_sub(de[:, :], E1[:, 0:CH:2], E0[:, 0:CH:2])
            L = pool.tile([128, 4 * Fh], F32, tag="ext")
            for blk in range(4):
                ls = slice(blk * Fh, (blk + 1) * Fh)
                nc.scalar.activation(L[:, ls], de[:, :], EXP_FN,
                                     bias=bias_ap(blk))
                nc.scalar.activation(L[:, ls], L[:, ls], LN_FN, bias=1.0)
            tt = pool.tile([128, 2 * Fh], F32, tag="yt")
            nc.vector.tensor_add(tt[:, 0:Fh], E0[:, 1:CH:2], E0[:, 0:CH:2])
            nc.vector.tensor_add(tt[:, Fh:2 * Fh], E1[:, 1:CH:2], E0[:, 0:CH:2])
            C1 = pool.tile([128, 4 * Fh], F32, tag="ca")
            for j in range(2):
                for i in range(2):
                    blk = 2 * j + i
                    nc.vector.tensor_add(C1[:, blk * Fh:(blk + 1) * Fh],
                                         L[:, blk * Fh:(blk + 1) * Fh],
                                         tt[:, j * Fh:(j + 1) * Fh])

            # ---- fixup: leaf 0 := identity, absorb alpha0 into t=1 leaf ----
            V = pool.tile([8, 4], F32, tag="vfix")
            nc.sync.dma_start(V[:, 0:2], E0[0:128:16, 0:2])
            nc.sync.dma_start(V[:, 2:4], E1[0:128:16, 0:2])
            Fx = pool.tile([8, 4], F32, tag="ffix")
            for j in range(2):
                for i in range(2):
                    blk = 2 * j + i
                    nc.vector.scalar_tensor_tensor(
                        Fx[:, blk:blk + 1], V[:, 2 * j + 1:2 * j + 2],
                        bias_ap(4 + blk, parts=8), V[:, 2 * i:2 * i + 1],
                        op0=add, op1=add)
            for blk in range(4):
                nc.sync.dma_start(C1[0:128:16, blk * Fh:blk * Fh + 1],
                                  Fx[:, blk:blk + 1])

            # ---- levels 2..NLVL: exp-domain log-semiring combine ----
            Cin = C1
            N = Fh
            ca_next = ["cb", "ca", "cb", "ca"]
            for lvl in range(2, NLVL + 1):
                F = N // 2
                co = 8 + 5 * (lvl - 2)
                Xt = pool.tile([128, 4 * F], F32, tag="xt")
                Yt = pool.tile([128, 4 * F], F32, tag="yt")
                EXt = pool.tile([128, 4 * F], F32, tag="ext")
                EYt = pool.tile([128, 4 * F], F32, tag="eyt")
                Ct = pool.tile([128, 4 * F], F32, tag=ca_next[lvl - 2])

                def blkap(t, blk, par, n=N):
                    off = blk * n
                    return t[:, off + par:off + n:2]

                for j in range(2):
                    for i in range(2):
                        blk = 2 * j + i
                        xs = slice(blk * F, (blk + 1) * F)
                        # X = A[j,0] + B[0,i]   (A odd, B even)
                        nc.vector.tensor_add(Xt[:, xs],
                                             blkap(Cin, 2 * j, 1),
                                             blkap(Cin, i, 0))
                        # Y = A[j,1] + B[1,i]
                        nc.vector.tensor_add(Yt[:, xs],
                                             blkap(Cin, 2 * j + 1, 1),
                                             blkap(Cin, 2 + i, 0))
                        # EY = exp(Y + bK - d)
                        nc.scalar.activation(EYt[:, xs], Yt[:, xs], EXP_FN,
                                             bias=bias_ap(co + blk))
                # EX = exp(X - d)
                nc.scalar.activation(EXt[:, 0:4 * F], Xt[:, 0:4 * F], EXP_FN,
                                     bias=bias_ap(co + 4))
                # S = EX + EY (in place on EXt)
                nc.vector.tensor_add(EXt[:, 0:4 * F], EXt[:, 0:4 * F],
                                     EYt[:, 0:4 * F])
                # C = ln(S)
                nc.scalar.activation(Ct[:, 0:4 * F], EXt[:, 0:4 * F], LN_FN,
                                     bias=0.0)
                Cin = Ct
                N = F

            nc.sync.dma_start(ptail_d.ap(), Cin[:, :])

    nc.compile()
    _BUILD_CACHE["nc"] = nc
    return nc


def _host_tail(ptails, K_final):
    """Finish the reduction in float64. ptails: [NCORE, 128, 4*FOUT]."""
    m = ptails.astype(np.float64).reshape(NCORE * BLOC, NCHUNK, 2, 2, FOUT)
    m = np.moveaxis(m, 4, 2).reshape(B, NCHUNK * FOUT, 2, 2)
    m = m + K_final[None, None]
    seq = m
    while seq.shape[1] > 1:
        n = seq.shape[1]
        carry = None
        if n % 2:
            carry = seq[:, -1:]
            seq = seq[:, :-1]
        Bm = seq[:, 0::2]   # earlier
        Am = seq[:, 1::2]   # later
        out = np.empty_like(Am)
        for j in range(2):
            for i in range(2):
                out[..., j, i] = np.logaddexp(
                    Am[..., j, 0] + Bm[..., 0, i],
                    Am[..., j, 1] + Bm[..., 1, i])
        if carry is not None:
            last = out[:, -1:]
            merged = np.empty_like(last)
            for j in range(2):
                for i in range(2):
                    merged[..., j, i] = np.logaddexp(
                        carry[..., j, 0] + last[..., 0, i],
                        carry[..., j, 1] + last[..., 1, i])
            out[:, -1:] = merged
        seq = out
    P = seq[:, 0]  # [B, 2, 2]
    LL = np.logaddexp.reduce(P.reshape(B, 4), axis=1)
    return -(LL.sum() / B)


def run(logits, mask, transitions, trace=False):
    logits = np.ascontiguousarray(np.asarray(logits), dtype=np.float32)
    tr = np.ascontiguousarray(np.asarray(transitions), dtype=np.float32)
    deltas = _estimate_deltas(logits, tr)
    cvals, K_final = _consts(tr, deltas)
    cst = np.ascontiguousarray(np.broadcast_to(cvals, (128, NCONST)))
    nc = _build()
    shards = logits.reshape(NCORE, BLOC, 2, T)
    in_maps = [{"logits": np.ascontiguousarray(shards[c]), "cst": cst}
               for c in range(NCORE)]
    res = run_bass_kernel_spmd(nc, in_maps, list(range(NCORE)), trace=trace)
    ptails = np.stack([res.results[c]["ptail"] for c in range(NCORE)])
    loss = np.float32(_host_tail(ptails, K_final))
    dec = np.concatenate([res.results[c]["decoded"] for c in range(NCORE)])
    decoded = dec.reshape(-1, 1).astype(np.int32)
    return (loss, decoded), res


def kernel(logits, mask, transitions):
    out, _ = run(logits, mask, transitions, trace=False)
    return out


# revision 18
# speedup vs baseline: 1.4453x; 1.0609x over previous
"""CRF forward-algorithm loss + argmax decode on 8 TRN2 NeuronCores.

Math: the CRF forward scan over T=65536 steps is a chain of 2x2
log-semiring matrix products (associative). Each core handles 8 batch
elements, computes the first NLVL levels of a parallel tree reduction
over the chain on-chip (emissions laid out [128 partitions x 4096]),
and exports the partial-product matrices. The host finishes the
reduction in float64 and assembles the loss. decoded is an elementwise
compare. alpha0 is absorbed into the t=1 leaf as a column offset so the
whole chain reduces to one logsumexp over the final 2x2 per batch.

Implementation notes (measured on HW):
- GpSimd elementwise runs concurrently with VectorE at ~1/3 speed and
  slows VectorE ~3x (shared SBUF ports) -> all TT work on VectorE,
  transcendentals on ScalarE (those coexist at full speed); GpSimd only
  issues SWDGE DMAs.
- Levels >= 2 combine in exp-domain: C = ln(exp(X-d) + exp(Y+bK-d)).
  bK is identical for all four matrix entries (row+col separability of
  the tracked offsets), so EX/EY/LN are one ACT instruction each.
- X/Y gather all four (j,i) combinations in ONE TensorTensor via
  zero-stride broadcast APs (verified exact + full rate on HW).
- exp-domain tiles are bf16: log-domain error ~4e-4 absolute/level,
  and the S=EX+EY add hits the DVE 2x bf16 mode.
- Work is split into 2 column segments, pipelined so segment 1's DMA
  loads overlap segment 0's compute.
- exp/ln share one ACT table set via a patched table-load pass.
- Transition-derived scalars arrive via the small "cst" input, so the
  compiled program is input-independent and cached across calls.
"""

import os
import sys
import types

import numpy as np

if "/opt/trn_rl_repo" not in sys.path:
    sys.path.insert(0, "/opt/trn_rl_repo")

import concourse.bacc as bacc
import concourse.bass as bass
import concourse.mybir as mybir
import concourse.tile as tile
from concourse.bass_types import AP
from concourse.bass_utils import run_bass_kernel_spmd

F32 = mybir.dt.float32
BF16 = mybir.dt.bfloat16
I32 = mybir.dt.int32

B, C, H, W = 64, 2, 256, 256
T = H * W                  # 65536
NCORE = 8
BLOC = B // NCORE          # 8 batch elements per core
NCHUNK = 16                # chunks per batch element -> 8*16 = 128 partitions
CH = T // NCHUNK           # 4096 t-positions per partition row
NLVL = 5                   # tree levels computed on chip
FOUT = CH >> NLVL          # level-NLVL positions per partition row exported
NSEG = 2
SEGW = CH // NSEG          # 2048 columns per segment

_BUILD_CACHE = {}

# consts layout: [0:4] level-1 exp biases D1[j][i], [4:8] fixup consts,
# then per level l=2..NLVL 2 entries: [bK-d (EY bias), -d (EX bias)].
NCONST = 8 + 2 * (NLVL - 1)


def _consts(tr, deltas):
    tr = np.asarray(tr, dtype=np.float64)
    cst = np.zeros(NCONST, np.float64)
    for j in range(2):
        for i in range(2):
            cst[2 * j + i] = (tr[j, 1] - tr[j, 0]) + (tr[1, i] - tr[0, i])
            cst[4 + 2 * j + i] = tr[j, i] - tr[j, 0] - tr[0, i]
    K = [[tr[j, 0] + tr[0, i] for i in range(2)] for j in range(2)]
    for lvl in range(2, NLVL + 1):
        off = 8 + 2 * (lvl - 2)
        d = float(deltas[lvl - 2])
        bK = (K[0][1] - K[0][0]) + (K[1][0] - K[0][0])
        cst[off] = bK - d
        cst[off + 1] = -d
        K = [[K[j][0] + K[0][i] + d for i in range(2)] for j in range(2)]
    return cst.astype(np.float32), np.array(K)


def _estimate_deltas(logits, tr):
    """Per-level centering constants from a small sample (float32 sim of
    the on-chip tree on the first 8 batch elements, 512 cols/partition)."""
    f32 = np.float32
    tr = np.asarray(tr, np.float64)
    lv = logits[0:BLOC].reshape(BLOC, 2, NCHUNK, CH)[:, :, :, 0:512]
    E0 = lv[:, 0].reshape(128, 512).astype(f32)
    E1 = lv[:, 1].reshape(128, 512).astype(f32)

    def sp(x):
        return np.logaddexp(0.0, x.astype(np.float64)).astype(f32)

    de = E1[:, 0::2] - E0[:, 0::2]
    Cb = {}
    K = [[tr[j, 0] + tr[0, i] for i in range(2)] for j in range(2)]
    for j in range(2):
        for i in range(2):
            D = (tr[j, 1] - tr[j, 0]) + (tr[1, i] - tr[0, i])
            t = (E0[:, 1::2] + E0[:, 0::2]) if j == 0 else (E1[:, 1::2] + E0[:, 0::2])
            Cb[(j, i)] = (t + sp(de + f32(D))).astype(f32)
    deltas = []
    for lvl in range(2, NLVL + 1):
        X00 = Cb[(0, 0)][:, 1::2] + Cb[(0, 0)][:, 0::2]
        d = float(np.round(X00.mean()))
        deltas.append(d)
        bK = (K[0][1] - K[0][0]) + (K[1][0] - K[0][0])
        Xn = {}
        for j in range(2):
            for i in range(2):
                Xv = (Cb[(j, 0)][:, 1::2] + Cb[(0, i)][:, 0::2]).astype(f32)
                Yv = (Cb[(j, 1)][:, 1::2] + Cb[(1, i)][:, 0::2]).astype(f32)
                S = np.exp(Xv - d) + np.exp(Yv + f32(bK - d))
                Xn[(j, i)] = np.log(S).astype(f32)
        K = [[K[j][0] + K[0][i] + d for i in range(2)] for j in range(2)]
        Cb = Xn
    return deltas


def _patched_act_tables(self):
    """insert_act_table_loads with Exp/Ln pinned to the one set that has
    both, so exp->ln sequences never thrash table loads."""
    import bass_rust
    from concourse.hw_specs import get_activation_tables
    EXP = mybir.ActivationFunctionType.Exp
    LN = mybir.ActivationFunctionType.Ln
    keep = "natural_log_exp_and_others"
    tables = [
        (name, funcs if name == keep
         else {f for f in funcs if f not in (EXP, LN)})
        for name, funcs in get_activation_tables(self.m.arch).items()
    ]
    bass_rust.insert_act_table_loads(self, tables)


def _xsel(cin, n, comp_off, parity):
    """Broadcast AP over cin [128, 4*n]: 4 blocks, block (j,i) reads
    component (comp_off==0: j-major row comps) at the given parity.

    comp_off=0: blocks (c00,c00,c10,c10); comp_off=1: (c01,c01,c11,c11).
    Returns AP with dims [part][2N,2(j)][0,2(rep i)][2,n/2] at parity.
    """
    base = cin[:, 0:4 * n]
    return AP(base.tensor, base.offset + comp_off * n + parity,
              [base.ap[0], [2 * n, 2], [0, 2], [2, n // 2]])


def _bsel(cin, n, row_off, parity):
    """row_off=0: blocks (c00,c01,c00,c01); row_off=2: (c10,c11,c10,c11)."""
    base = cin[:, 0:4 * n]
    return AP(base.tensor, base.offset + row_off * n + parity,
              [base.ap[0], [0, 2], [n, 2], [2, n // 2]])


def _build():
    if "nc" in _BUILD_CACHE:
        return _BUILD_CACHE["nc"]

    nc = bacc.Bacc("TRN2", target_bir_lowering=False, debug=False,
                   num_devices=NCORE)
    nc.insert_act_table_loads = types.MethodType(_patched_act_tables, nc)
    logits_d = nc.dram_tensor("logits", [BLOC, 2, T], F32, kind="ExternalInput")
    cst_d = nc.dram_tensor("cst", [128, NCONST], F32, kind="ExternalInput")
    dec_d = nc.dram_tensor("decoded", [BLOC, T], I32, kind="ExternalOutput")
    ptail_d = nc.dram_tensor("ptail", [128, 4 * FOUT], F32, kind="ExternalOutput")

    gt = mybir.AluOpType.is_gt
    add = mybir.AluOpType.add
    EXP_FN = mybir.ActivationFunctionType.Exp
    LN_FN = mybir.ActivationFunctionType.Ln

    with tile.TileContext(nc) as tc:
        with tc.tile_pool(name="main", bufs=1) as pool:
            cst = pool.tile([128, NCONST], F32, tag="cst")
            nc.sync.dma_start(cst[:, :], cst_d.ap())

            def bias_ap(k, parts=128):
                return cst[0:parts, k:k + 1]

            src = logits_d.ap().rearrange("b c (k f) -> b c k f", f=CH)
            ddst = dec_d.ap().rearrange("b (k f) -> b k f", f=CH)
            pt4 = ptail_d.ap().rearrange("p (blk s f) -> p blk s f",
                                         s=NSEG, f=FOUT // NSEG)

            for seg in range(NSEG):
                cs = slice(seg * SEGW, (seg + 1) * SEGW)
                E0 = pool.tile([128, SEGW], F32, tag=f"e0_{seg}")
                E1 = pool.tile([128, SEGW], F32, tag=f"e1_{seg}")
                if seg == 0:
                    nc.sync.dma_start(E0[:, :], src[:, 0, :, cs])
                    nc.scalar.dma_start(E1[:, :], src[:, 1, :, cs])
                else:
                    nc.gpsimd.dma_start(E0[:, :], src[:, 0, :, cs])
                    nc.gpsimd.dma_start(E1[:, :], src[:, 1, :, cs])

                # decoded = (e1 > e0)
                dec = pool.tile([128, SEGW], I32, tag=f"dec_{seg}")
                nc.vector.tensor_tensor(dec[:, :], E1[:, :], E0[:, :], op=gt)
                nc.sync.dma_start(ddst[:, :, cs], dec[:, :])

                # ---- level 1 ----
                Fh = SEGW // 2  # 1024
                de = pool.tile([128, Fh], F32, tag=f"de_{seg}")
                nc.vector.tensor_sub(de[:, :], E1[:, 0:SEGW:2], E0[:, 0:SEGW:2])
                L = pool.tile([128, 4 * Fh], F32, tag=f"l_{seg}")
                # D1[0][1] == D1[1][0]: merge middle blocks into one ACT
                nc.scalar.activation(L[:, 0:Fh], de[:, :], EXP_FN,
                                     bias=bias_ap(0))
                deap = de[:, :]
                de2 = AP(deap.tensor, deap.offset,
                         [deap.ap[0], [0, 2], [1, Fh]])
                nc.scalar.activation(L[:, Fh:3 * Fh], de2, EXP_FN,
                                     bias=bias_ap(1))
                nc.scalar.activation(L[:, 3 * Fh:4 * Fh], de[:, :], EXP_FN,
                                     bias=bias_ap(3))
                nc.scalar.activation(L[:, :], L[:, :], LN_FN, bias=1.0)
                tt = pool.tile([128, 2 * Fh], F32, tag=f"tt_{seg}")
                nc.vector.tensor_add(tt[:, 0:Fh], E0[:, 1:SEGW:2],
                                     E0[:, 0:SEGW:2])
                nc.vector.tensor_add(tt[:, Fh:2 * Fh], E1[:, 1:SEGW:2],
                                     E0[:, 0:SEGW:2])
                C1 = pool.tile([128, 4 * Fh], F32, tag=f"ca_{seg}")
                ttap = tt[:, :]
                tsel = AP(ttap.tensor, ttap.offset,
                          [ttap.ap[0], [Fh, 2], [0, 2], [1, Fh]])
                nc.vector.tensor_add(C1[:, :], L[:, :], tsel)

                if seg == 0:
                    # fixup: leaf 0 := identity; alpha0 -> t=1 leaf column
                    V = pool.tile([8, 4], F32, tag="vfix")
                    nc.sync.dma_start(V[:, 0:2], E0[0:128:16, 0:2])
                    nc.sync.dma_start(V[:, 2:4], E1[0:128:16, 0:2])
                    Fx = pool.tile([8, 4], F32, tag="ffix")
                    for j in range(2):
                        for i in range(2):
                            blk = 2 * j + i
                            nc.vector.scalar_tensor_tensor(
                                Fx[:, blk:blk + 1],
                                V[:, 2 * j + 1:2 * j + 2],
                                bias_ap(4 + blk, parts=8),
                                V[:, 2 * i:2 * i + 1],
                                op0=add, op1=add)
                    for blk in range(4):
                        nc.sync.dma_start(
                            C1[0:128:16, blk * Fh:blk * Fh + 1],
                            Fx[:, blk:blk + 1])

                # ---- levels 2..NLVL: exp-domain combine ----
                Cin = C1
                N = Fh
                nxt = ["cb", "ca", "cb", "ca"]
                for lvl in range(2, NLVL + 1):
                    F = N // 2
                    co = 8 + 2 * (lvl - 2)
                    Xt = pool.tile([128, 4 * F], F32, tag=f"xt_{seg}")
                    Yt = pool.tile([128, 4 * F], F32, tag=f"yt_{seg}")
                    EXt = pool.tile([128, 4 * F], BF16, tag=f"ext_{seg}")
                    EYt = pool.tile([128, 4 * F], BF16, tag=f"eyt_{seg}")
                    Ct = pool.tile([128, 4 * F], F32,
                                   tag=f"{nxt[lvl - 2]}_{seg}")
                    # X = A[j,0] + B[0,i] over all 4 blocks in one instr
                    nc.vector.tensor_tensor(Xt[:, :], _xsel(Cin, N, 0, 1),
                                            _bsel(Cin, N, 0, 0), op=add)
                    # Y = A[j,1] + B[1,i]
                    nc.vector.tensor_tensor(Yt[:, :], _xsel(Cin, N, 1, 1),
                                            _bsel(Cin, N, 2, 0), op=add)
                    # EX = exp(X - d); EY = exp(Y + bK - d)
                    nc.scalar.activation(EXt[:, :], Xt[:, :], EXP_FN,
                                         bias=bias_ap(co + 1))
                    nc.scalar.activation(EYt[:, :], Yt[:, :], EXP_FN,
                                         bias=bias_ap(co))
                    # S = EX + EY (bf16 2x mode), in place on EXt
                    nc.vector.tensor_add(EXt[:, :], EXt[:, :], EYt[:, :])
                    # C = ln(S)
                    nc.scalar.activation(Ct[:, :], EXt[:, :], LN_FN, bias=0.0)
                    Cin = Ct
                    N = F

                nc.sync.dma_start(pt4[:, :, seg, :],
                                  Cin[:, :].rearrange("p (blk f) -> p blk f",
                                                      f=FOUT // NSEG))

    nc.compile()
    _BUILD_CACHE["nc"] = nc
    return nc


def _host_tail(ptails, K_final):
    """Finish the reduction in float64. ptails: [NCORE, 128, 4*FOUT]."""
    m = ptails.astype(np.float64).reshape(NCORE * BLOC, NCHUNK, 2, 2, FOUT)
    m = np.moveaxis(m, 4, 2).reshape(B, NCHUNK * FOUT, 2, 2)
    m = m + K_final[None, None]
    seq = m
    while seq.shape[1] > 1:
        n = seq.shape[1]
        carry = None
        if n % 2:
            carry = seq[:, -1:]
            seq = seq[:, :-1]
        Bm = seq[:, 0::2]   # earlier
        Am = seq[:, 1::2]   # later
        out = np.empty_like(Am)
        for j in range(2):
            for i in range(2):
                out[..., j, i] = np.logaddexp(
                    Am[..., j, 0] + Bm[..., 0, i],
                    Am[..., j, 1] + Bm[..., 1, i])
        if carry is not None:
            last = out[:, -1:]
            merged = np.empty_like(last)
            for j in range(2):
                for i in range(2):
                    merged[..., j, i] = np.logaddexp(
                        carry[..., j, 0] + last[..., 0, i],
                        carry[..., j, 1] + last[..., 1, i])
            out[:, -1:] = merged
        seq = out
    P = seq[:, 0]  # [B, 2, 2]
    LL = np.logaddexp.reduce(P.reshape(B, 4), axis=1)
    return -(LL.sum() / B)


def run(logits, mask, transitions, trace=False):
    logits = np.ascontiguousarray(np.asarray(logits), dtype=np.float32)
    tr = np.ascontiguousarray(np.asarray(transitions), dtype=np.float32)
    deltas = _estimate_deltas(logits, tr)
    cvals, K_final = _consts(tr, deltas)
    cst = np.ascontiguousarray(np.broadcast_to(cvals, (128, NCONST)))
    nc = _build()
    shards = logits.reshape(NCORE, BLOC, 2, T)
    in_maps = [{"logits": np.ascontiguousarray(shards[c]), "cst": cst}
               for c in range(NCORE)]
    res = run_bass_kernel_spmd(nc, in_maps, list(range(NCORE)), trace=trace)
    ptails = np.stack([res.results[c]["ptail"] for c in range(NCORE)])
    loss = np.float32(_host_tail(ptails, K_final))
    dec = np.concatenate([res.results[c]["decoded"] for c in range(NCORE)])
    decoded = dec.reshape(-1, 1).astype(np.int32)
    return (loss, decoded), res


def kernel(logits, mask, transitions):
    out, _ = run(logits, mask, transitions, trace=False)
    return out


# revision 19
# speedup vs baseline: 1.9815x; 1.3710x over previous
"""CRF forward-algorithm loss + argmax decode on 8 TRN2 NeuronCores.

Math: the CRF forward scan over T=65536 steps is a chain of 2x2
log-semiring matrix products (associative). Each core handles 8 batch
elements, computes the first NLVL levels of a parallel tree reduction
over the chain on-chip (emissions laid out [128 partitions x 4096]),
and exports the partial-product matrices. The host finishes the
reduction in float64 and assembles the loss. decoded is an elementwise
compare. alpha0 is absorbed into the t=1 leaf as a column offset so the
whole chain reduces to one logsumexp over the final 2x2 per batch.

Implementation notes (measured on HW):
- GpSimd elementwise runs concurrently with VectorE at ~1/3 speed and
  slows VectorE ~3x (shared SBUF ports) -> all TT work on VectorE,
  transcendentals on ScalarE (those coexist at full speed); GpSimd only
  issues SWDGE DMAs.
- Levels >= 2 combine in exp-domain: C = ln(exp(X-d) + exp(Y+bK-d)).
  bK is identical for all four matrix entries (row+col separability of
  the tracked offsets), so EX/EY/LN are one ACT instruction each.
- X/Y gather all four (j,i) combinations in ONE TensorTensor via
  zero-stride broadcast APs (verified exact + full rate on HW).
- exp-domain tiles are bf16: log-domain error ~4e-4 absolute/level,
  and the S=EX+EY add hits the DVE 2x bf16 mode.
- Work is split into 2 column segments, pipelined so segment 1's DMA
  loads overlap segment 0's compute.
- exp/ln share one ACT table set via a patched table-load pass.
- Transition-derived scalars arrive via the small "cst" input, so the
  compiled program is input-independent and cached across calls.
"""

import os
import sys
import types

import numpy as np

if "/opt/trn_rl_repo" not in sys.path:
    sys.path.insert(0, "/opt/trn_rl_repo")

import concourse.bacc as bacc
import concourse.bass as bass
import concourse.mybir as mybir
import concourse.tile as tile
from concourse.bass_types import AP
from concourse.bass_utils import run_bass_kernel_spmd

F32 = mybir.dt.float32
BF16 = mybir.dt.bfloat16
I32 = mybir.dt.int32

B, C, H, W = 64, 2, 256, 256
T = H * W                  # 65536
NCORE = 8
BLOC = B // NCORE          # 8 batch elements per core
NCHUNK = 16                # chunks per batch element -> 8*16 = 128 partitions
CH = T // NCHUNK           # 4096 t-positions per partition row
NLVL = 5                   # tree levels computed on chip
FOUT = CH >> NLVL          # level-NLVL positions per partition row exported
NSEG = 2
SEGW = CH // NSEG          # 2048 columns per segment

_BUILD_CACHE = {}

# consts layout: [0:4] level-1 exp biases D1[j][i], [4:8] fixup consts,
# then per level l=2..NLVL 2 entries: [bK-d (EY bias), -d (EX bias)].
NCONST = 8 + 2 * (NLVL - 1)


def _consts(tr, deltas):
    tr = np.asarray(tr, dtype=np.float64)
    cst = np.zeros(NCONST, np.float64)
    for j in range(2):
        for i in range(2):
            cst[2 * j + i] = (tr[j, 1] - tr[j, 0]) + (tr[1, i] - tr[0, i])
            cst[4 + 2 * j + i] = tr[j, i] - tr[j, 0] - tr[0, i]
    K = [[tr[j, 0] + tr[0, i] for i in range(2)] for j in range(2)]
    for lvl in range(2, NLVL + 1):
        off = 8 + 2 * (lvl - 2)
        d = float(deltas[lvl - 2])
        bK = (K[0][1] - K[0][0]) + (K[1][0] - K[0][0])
        cst[off] = bK - d
        cst[off + 1] = -d
        K = [[K[j][0] + K[0][i] + d for i in range(2)] for j in range(2)]
    return cst.astype(np.float32), np.array(K)


def _estimate_deltas(logits, tr):
    """Per-level centering constants from a small sample (float32 sim of
    the on-chip tree on the first 8 batch elements, 512 cols/partition)."""
    f32 = np.float32
    tr = np.asarray(tr, np.float64)
    lv = logits[0:BLOC].reshape(BLOC, 2, NCHUNK, CH)[:, :, :, 0:512]
    E0 = lv[:, 0].reshape(128, 512).astype(f32)
    E1 = lv[:, 1].reshape(128, 512).astype(f32)

    def sp(x):
        return np.logaddexp(0.0, x.astype(np.float64)).astype(f32)

    de = E1[:, 0::2] - E0[:, 0::2]
    Cb = {}
    K = [[tr[j, 0] + tr[0, i] for i in range(2)] for j in range(2)]
    for j in range(2):
        for i in range(2):
            D = (tr[j, 1] - tr[j, 0]) + (tr[1, i] - tr[0, i])
            t = (E0[:, 1::2] + E0[:, 0::2]) if j == 0 else (E1[:, 1::2] + E0[:, 0::2])
            Cb[(j, i)] = (t + sp(de + f32(D))).astype(f32)
    deltas = []
    for lvl in range(2, NLVL + 1):
        X00 = Cb[(0, 0)][:, 1::2] + Cb[(0, 0)][:, 0::2]
        d = float(np.round(X00.mean()))
        deltas.append(d)
        bK = (K[0][1] - K[0][0]) + (K[1][0] - K[0][0])
        Xn = {}
        for j in range(2):
            for i in range(2):
                Xv = (Cb[(j, 0)][:, 1::2] + Cb[(0, i)][:, 0::2]).astype(f32)
                Yv = (Cb[(j, 1)][:, 1::2] + Cb[(1, i)][:, 0::2]).astype(f32)
                S = np.exp(Xv - d) + np.exp(Yv + f32(bK - d))
                Xn[(j, i)] = np.log(S).astype(f32)
        K = [[K[j][0] + K[0][i] + d for i in range(2)] for j in range(2)]
        Cb = Xn
    return deltas


def _patched_act_tables(self):
    """insert_act_table_loads with Exp/Ln pinned to the one set that has
    both, so exp->ln sequences never thrash table loads."""
    import bass_rust
    from concourse.hw_specs import get_activation_tables
    EXP = mybir.ActivationFunctionType.Exp
    LN = mybir.ActivationFunctionType.Ln
    keep = "natural_log_exp_and_others"
    tables = [
        (name, funcs if name == keep
         else {f for f in funcs if f not in (EXP, LN)})
        for name, funcs in get_activation_tables(self.m.arch).items()
    ]
    bass_rust.insert_act_table_loads(self, tables)


def _xsel(cin, n, comp_off, parity):
    """Broadcast AP over cin [128, 4*n]: 4 blocks, block (j,i) reads
    component (comp_off==0: j-major row comps) at the given parity.

    comp_off=0: blocks (c00,c00,c10,c10); comp_off=1: (c01,c01,c11,c11).
    Returns AP with dims [part][2N,2(j)][0,2(rep i)][2,n/2] at parity.
    """
    base = cin[:, 0:4 * n]
    return AP(base.tensor, base.offset + comp_off * n + parity,
              [base.ap[0], [2 * n, 2], [0, 2], [2, n // 2]])


def _bsel(cin, n, row_off, parity):
    """row_off=0: blocks (c00,c01,c00,c01); row_off=2: (c10,c11,c10,c11)."""
    base = cin[:, 0:4 * n]
    return AP(base.tensor, base.offset + row_off * n + parity,
              [base.ap[0], [0, 2], [n, 2], [2, n // 2]])


def _build():
    if "nc" in _BUILD_CACHE:
        return _BUILD_CACHE["nc"]

    nc = bacc.Bacc("TRN2", target_bir_lowering=False, debug=False,
                   num_devices=NCORE)
    nc.insert_act_table_loads = types.MethodType(_patched_act_tables, nc)
    # host pre-interleaves logits into SBUF layout: row p=(b,chunk) holds
    # [E0 seg0 | E1 seg0 | E0 seg1 | E1 seg1], 16KB contiguous per segment
    # per partition row (large DMA descriptors -> near-peak HBM rate).
    logits_d = nc.dram_tensor("logits", [128, 2 * CH], F32, kind="ExternalInput")
    cst_d = nc.dram_tensor("cst", [128, NCONST], F32, kind="ExternalInput")
    dec_d = nc.dram_tensor("decoded", [BLOC, T], I32, kind="ExternalOutput")
    ptail_ds = [nc.dram_tensor(f"ptail{s}", [128, 4 * (FOUT // NSEG)], F32,
                               kind="ExternalOutput") for s in range(NSEG)]

    gt = mybir.AluOpType.is_gt
    add = mybir.AluOpType.add
    EXP_FN = mybir.ActivationFunctionType.Exp
    LN_FN = mybir.ActivationFunctionType.Ln

    with tile.TileContext(nc) as tc:
        with tc.tile_pool(name="main", bufs=1) as pool:
            cst = pool.tile([128, NCONST], F32, tag="cst")
            nc.sync.dma_start(cst[:, :], cst_d.ap())

            def bias_ap(k, parts=128):
                return cst[0:parts, k:k + 1]

            src = logits_d.ap()
            ddst = dec_d.ap().rearrange("b (k f) -> b k f", f=CH)

            for seg in range(NSEG):
                cs = slice(seg * SEGW, (seg + 1) * SEGW)
                Es = pool.tile([128, 2 * SEGW], F32, tag=f"es_{seg}")
                issuer = nc.sync if seg == 0 else nc.scalar
                issuer.dma_start(Es[:, :],
                                 src[:, seg * 2 * SEGW:(seg + 1) * 2 * SEGW])
                E0 = Es[:, 0:SEGW]
                E1 = Es[:, SEGW:2 * SEGW]

                # decoded = (e1 > e0)
                dec = pool.tile([128, SEGW], I32, tag=f"dec_{seg}")
                nc.vector.tensor_tensor(dec[:, :], E1, E0, op=gt)
                nc.gpsimd.dma_start(ddst[:, :, cs], dec[:, :])

                # ---- level 1 ----
                Fh = SEGW // 2  # 1024
                de = pool.tile([128, Fh], F32, tag=f"de_{seg}")
                nc.vector.tensor_sub(de[:, :], Es[:, SEGW:2 * SEGW:2],
                                     Es[:, 0:SEGW:2])
                L = pool.tile([128, 4 * Fh], F32, tag=f"l_{seg}")
                # D1[0][1] == D1[1][0]: merge middle blocks into one ACT
                nc.scalar.activation(L[:, 0:Fh], de[:, :], EXP_FN,
                                     bias=bias_ap(0))
                deap = de[:, :]
                de2 = AP(deap.tensor, deap.offset,
                         [deap.ap[0], [0, 2], [1, Fh]])
                nc.scalar.activation(L[:, Fh:3 * Fh], de2, EXP_FN,
                                     bias=bias_ap(1))
                nc.scalar.activation(L[:, 3 * Fh:4 * Fh], de[:, :], EXP_FN,
                                     bias=bias_ap(3))
                nc.scalar.activation(L[:, :], L[:, :], LN_FN, bias=1.0)
                tt = pool.tile([128, 2 * Fh], F32, tag=f"tt_{seg}")
                nc.vector.tensor_add(tt[:, 0:Fh], Es[:, 1:SEGW:2],
                                     Es[:, 0:SEGW:2])
                nc.vector.tensor_add(tt[:, Fh:2 * Fh],
                                     Es[:, SEGW + 1:2 * SEGW:2],
                                     Es[:, 0:SEGW:2])
                C1 = pool.tile([128, 4 * Fh], F32, tag=f"ca_{seg}")
                ttap = tt[:, :]
                tsel = AP(ttap.tensor, ttap.offset,
                          [ttap.ap[0], [Fh, 2], [0, 2], [1, Fh]])
                nc.vector.tensor_add(C1[:, :], L[:, :], tsel)

                if seg == 0:
                    # fixup: leaf 0 := identity; alpha0 -> t=1 leaf column
                    V = pool.tile([8, 4], F32, tag="vfix")
                    nc.sync.dma_start(V[:, 0:2], Es[0:128:16, 0:2])
                    nc.sync.dma_start(V[:, 2:4], Es[0:128:16, SEGW:SEGW + 2])
                    Fx = pool.tile([8, 4], F32, tag="ffix")
                    for j in range(2):
                        for i in range(2):
                            blk = 2 * j + i
                            nc.vector.scalar_tensor_tensor(
                                Fx[:, blk:blk + 1],
                                V[:, 2 * j + 1:2 * j + 2],
                                bias_ap(4 + blk, parts=8),
                                V[:, 2 * i:2 * i + 1],
                                op0=add, op1=add)
                    for blk in range(4):
                        nc.sync.dma_start(
                            C1[0:128:16, blk * Fh:blk * Fh + 1],
                            Fx[:, blk:blk + 1])

                # ---- levels 2..NLVL: exp-domain combine ----
                Cin = C1
                N = Fh
                nxt = ["cb", "ca", "cb", "ca"]
                for lvl in range(2, NLVL + 1):
                    F = N // 2
                    co = 8 + 2 * (lvl - 2)
                    Xt = pool.tile([128, 4 * F], F32, tag=f"xt_{seg}")
                    Yt = pool.tile([128, 4 * F], F32, tag=f"yt_{seg}")
                    EXt = pool.tile([128, 4 * F], BF16, tag=f"ext_{seg}")
                    EYt = pool.tile([128, 4 * F], BF16, tag=f"eyt_{seg}")
                    Ct = pool.tile([128, 4 * F], F32,
                                   tag=f"{nxt[lvl - 2]}_{seg}")
                    # X = A[j,0] + B[0,i] over all 4 blocks in one instr
                    nc.vector.tensor_tensor(Xt[:, :], _xsel(Cin, N, 0, 1),
                                            _bsel(Cin, N, 0, 0), op=add)
                    # Y = A[j,1] + B[1,i]
                    nc.vector.tensor_tensor(Yt[:, :], _xsel(Cin, N, 1, 1),
                                            _bsel(Cin, N, 2, 0), op=add)
                    # EX = exp(X - d); EY = exp(Y + bK - d)
                    nc.scalar.activation(EXt[:, :], Xt[:, :], EXP_FN,
                                         bias=bias_ap(co + 1))
                    nc.scalar.activation(EYt[:, :], Yt[:, :], EXP_FN,
                                         bias=bias_ap(co))
                    # S = EX + EY (bf16 2x mode), in place on EXt
                    nc.vector.tensor_add(EXt[:, :], EXt[:, :], EYt[:, :])
                    # C = ln(S)
                    nc.scalar.activation(Ct[:, :], EXt[:, :], LN_FN, bias=0.0)
                    Cin = Ct
                    N = F

                nc.sync.dma_start(ptail_ds[seg].ap(), Cin[:, :])

    nc.compile()
    _BUILD_CACHE["nc"] = nc
    return nc


def _host_tail(ptails, K_final):
    """Finish the reduction in float64. ptails: [NCORE, 128, 4*FOUT]."""
    m = ptails.astype(np.float64).reshape(NCORE * BLOC, NCHUNK, 2, 2, FOUT)
    m = np.moveaxis(m, 4, 2).reshape(B, NCHUNK * FOUT, 2, 2)
    m = m + K_final[None, None]
    seq = m
    while seq.shape[1] > 1:
        n = seq.shape[1]
        carry = None
        if n % 2:
            carry = seq[:, -1:]
            seq = seq[:, :-1]
        Bm = seq[:, 0::2]   # earlier
        Am = seq[:, 1::2]   # later
        out = np.empty_like(Am)
        for j in range(2):
            for i in range(2):
                out[..., j, i] = np.logaddexp(
                    Am[..., j, 0] + Bm[..., 0, i],
                    Am[..., j, 1] + Bm[..., 1, i])
        if carry is not None:
            last = out[:, -1:]
            merged = np.empty_like(last)
            for j in range(2):
                for i in range(2):
                    merged[..., j, i] = np.logaddexp(
                        carry[..., j, 0] + last[..., 0, i],
                        carry[..., j, 1] + last[..., 1, i])
            out[:, -1:] = merged
        seq = out
    P = seq[:, 0]  # [B, 2, 2]
    LL = np.logaddexp.reduce(P.reshape(B, 4), axis=1)
    return -(LL.sum() / B)


def run(logits, mask, transitions, trace=False):
    logits = np.ascontiguousarray(np.asarray(logits), dtype=np.float32)
    tr = np.ascontiguousarray(np.asarray(transitions), dtype=np.float32)
    deltas = _estimate_deltas(logits, tr)
    cvals, K_final = _consts(tr, deltas)
    cst = np.ascontiguousarray(np.broadcast_to(cvals, (128, NCONST)))
    nc = _build()
    # [b, c2, k, seg, f] -> [(b k), seg, c2, f]
    shards = logits.reshape(NCORE, BLOC, 2, NCHUNK, NSEG, SEGW)
    shards = shards.transpose(0, 1, 3, 4, 2, 5).reshape(NCORE, 128, 2 * CH)
    in_maps = [{"logits": np.ascontiguousarray(shards[c]), "cst": cst}
               for c in range(NCORE)]
    res = run_bass_kernel_spmd(nc, in_maps, list(range(NCORE)), trace=trace)
    # reassemble [core, 128, 4*FOUT] with position index seg*64+f per block
    ptails = np.stack([
        np.concatenate([res.results[c][f"ptail{s}"].reshape(128, 4,
                                                            FOUT // NSEG)
                        for s in range(NSEG)], axis=2).reshape(128, 4 * FOUT)
        for c in range(NCORE)])
    loss = np.float32(_host_tail(ptails, K_final))
    dec = np.concatenate([res.results[c]["decoded"] for c in range(NCORE)])
    decoded = dec.reshape(-1, 1).astype(np.int32)
    return (loss, decoded), res


def kernel(logits, mask, transitions):
    out, _ = run(logits, mask, transitions, trace=False)
    return out
